# revision 23
# baseline (speedup 1.0000x reference)
"""Trainium2 Bass kernel for nn_MultiHeadDotProductAttention_14980845928960.

Block-local multi-head attention with partial RoPE:
  q/k/v projections -> RoPE on first 32 of 64 head dims -> softmax(QK^T/8)V
  -> output projection.  Shapes: inputs [4,16,256,1024], 16 heads x 64 dim,
  blocks of 256 tokens attend locally.

Strategy: data-parallel over the 64 (batch, block) pairs -> 8 blocks/core,
processed as 4 pairs of blocks (512 tokens each).
  - x^T prepared HOST-side (transpose + bf16 cast) so no PE transposes or
    their PSUM evacuations are needed; DMA traffic for x halves.
  - Q/K channel-PERMUTED (host side) so rope dims occupy out-chunks 0-3
    and pass dims chunks 4-7; RoPE = R-matmul (pair swap w/ signs) + two
    elementwise multiplies with cos/sin tables; the R-matmul is emitted
    one chunk late (software pipeline) so the PE never waits on the DVE
    evacuation it consumes.
  - scores computed TRANSPOSED (k on partitions); both k-chunks of one
    head share a single [128,512] PSUM bank so ONE exp per head runs on
    ScalarE; softmax needs no max-subtraction (scores ~N(0,1)).
  - PV packs two heads per PSUM bank; denominators arrive replicated on
    PV-output partitions 64:128 via v_aug = [v_h | 1 x64]; reciprocal is
    exp(-ln(x)) on ScalarE -- both funcs live in ONE activation table so
    no ACT_TABLE_LOAD thrash; normalization folds into the PSUM->SBUF
    evacuation on DVE.
  - biases are all-zero per the problem spec; the fast path elides the
    bias adds entirely (a bias-capable slow path is kept for safety).
  - compute dtype bf16 on the PE, fp32 PSUM accumulate; output written
    bf16 and cast to f32 on host.
  - per-pair input DMA is prefetched one pair ahead on separate queues.
"""

import ml_dtypes
import numpy as np

import concourse.bass as bass
import concourse.tile as tile
from concourse import mybir
from concourse.bass_utils import run_bass_kernel_spmd

# ---------------------------------------------------------------- constants
B, NB, BS, F = 4, 16, 256, 1024
H, D, ROPE = 16, 64, 32
NCORES = 8
BLKS = B * NB                 # 64 blocks total
BPC = BLKS // NCORES          # 8 blocks per core
NPAIR = BPC // 2              # block pairs per core
BT = 2 * BS                   # tokens per pair (512)
TOK = BPC * BS                # tokens per core (2048)
F32 = mybir.dt.float32
BF16 = mybir.dt.bfloat16
WDT = BF16
WNP = ml_dtypes.bfloat16
MULT = mybir.AluOpType.mult
ADD = mybir.AluOpType.add
EXP = mybir.ActivationFunctionType.Exp
LN = mybir.ActivationFunctionType.Ln

# ------------------------------------------------- walrus multi-wait splitter
# This walrus build rejects >1 sync-wait per instruction on several
# instruction structs. Tile attaches several waits to one instruction;
# hoist extras onto NOPs inserted just before it on the same engine.
_split_ctr = [0]


def _split_multi_waits(nc, maxw=1):
    for f in nc.m.functions:
        for bb in f.blocks:
            insts = list(bb.instructions)
            out = []
            changed = False
            for inst in insts:
                si = inst.sync_info
                waits = list(si.on_wait) if si and si.on_wait else []
                if len(waits) > maxw:
                    changed = True
                    for w in waits[:-maxw]:
                        _split_ctr[0] += 1
                        nop = mybir.InstNoOp(
                            name=f"wsplit-{_split_ctr[0]}",
                            ins=[],
                            outs=[],
                            engine=inst.engine,
                        )
                        nop.sync_info = mybir.SyncInfo(on_wait=[w], on_update=[])
                        nc.register_instruction(nop)
                        out.append(nop)
                    si.on_wait = waits[-maxw:]
                out.append(inst)
            if changed:
                bb.instructions = out


# ---------------------------------------------------------------- bass build
class _Ctx:
    """Per-program emission state (pools, weight tiles, dram handles)."""

    def __init__(self, nc, use_bias):
        self.nc = nc
        self.use_bias = use_bias


def _bank(ctx):
    """Allocate a full PSUM bank [128, 512] f32 from the shared FIFO ring."""
    return ctx.psum.tile([128, BT], F32, tag="bank", name="bank")


def _emit_x_dma(ctx, pair, prologue=False):
    """Issue x^T chunk loads + cos/sin tables for one pair (prefetch).

    In the prologue the first pair's xq is split across the gpsimd and
    scalar queues (wq owns the sync queue) so the first projection can
    start as early as possible.
    """
    nc = ctx.nc
    xq_t, xkv_t = [], []
    sl = slice(pair * BT, (pair + 1) * BT)
    for c in range(8):
        t = ctx.xtp.tile([128, BT], WDT, tag=f"xq{c}", name=f"xq{c}", bufs=2)
        eng = (nc.gpsimd if c % 2 == 0 else nc.scalar) if prologue else nc.gpsimd
        eng.dma_start(out=t, in_=ctx.xq_d[c][:, sl])
        xq_t.append(t)
    for c in range(8):
        t = ctx.xtp.tile([128, BT], WDT, tag=f"xk{c}", name=f"xk{c}", bufs=2)
        nc.sync.dma_start(out=t, in_=ctx.xkv_d[c][:, sl])
        xkv_t.append(t)
    cos_sb = ctx.tabp.tile([128, BT], F32, tag="cos", name="cos", bufs=2)
    nc.scalar.dma_start(out=cos_sb, in_=ctx.cos_d[pair])
    sin_sb = ctx.tabp.tile([128, BT], F32, tag="sin", name="sin", bufs=2)
    nc.scalar.dma_start(out=sin_sb, in_=ctx.sin_d[pair])
    return dict(xq=xq_t, xkv=xkv_t, cos=cos_sb, sin=sin_sb)


def _proj_chunk(ctx, w_sb, b_sb, x_tiles, tagpfx, oc, io, ocmaj=False):
    """One 128-chan projection chunk: 8 accumulating matmuls + evacuation.

    Returns (qf_tile, rope_tail).  For rope chunks (oc<4) rope_tail is a
    closure finishing RoPE (R-matmul + cos/sin combines); the caller
    emits it one chunk later so the PE never waits on the DVE copy.
    """
    nc = ctx.nc
    ps = _bank(ctx)
    for c in range(8):
        # wq is host-re-blocked oc-major so chunk oc's weights arrive as
        # one early 256KB tile instead of 1/8th of every 2MB of wq.
        lhsT = (w_sb[oc][:, c * 128 : (c + 1) * 128] if ocmaj
                else w_sb[c][:, oc * 128 : (oc + 1) * 128])
        nc.tensor.matmul(
            ps,
            lhsT=lhsT,
            rhs=x_tiles[c],
            start=(c == 0),
            stop=(c == 7),
        )
    qf = ctx.qk.tile([128, BT], WDT, tag=f"{tagpfx}{oc}", name=f"{tagpfx}{oc}")
    if oc >= 4:
        # pass chunk: plain evacuation (bias add only on slow path).
        # GPSIMD cannot read PSUM, so this stays on DVE.
        if ctx.use_bias:
            nc.vector.tensor_scalar_add(qf, ps, b_sb[:, oc : oc + 1])
        else:
            nc.vector.tensor_copy(out=qf, in_=ps)
        return qf, None

    raw = ctx.qk.tile([128, BT], WDT, tag=f"{tagpfx}r{oc}", name=f"{tagpfx}r{oc}")
    if ctx.use_bias:
        nc.vector.tensor_scalar_add(raw, ps, b_sb[:, oc : oc + 1])
    else:
        nc.vector.tensor_copy(out=raw, in_=ps)

    def rope_tail():
        ps2 = _bank(ctx)
        nc.tensor.matmul(ps2, lhsT=ctx.rt_sb, rhs=raw, start=True, stop=True)
        qs2 = ctx.qk.tile([128, BT], F32, tag="qs2", name="qs2", bufs=2)
        nc.vector.tensor_tensor(out=qs2, in0=ps2, in1=io["sin"], op=MULT)
        nc.gpsimd.tensor_tensor(out=qf, in0=raw, in1=io["cos"], op=MULT)
        nc.gpsimd.tensor_tensor(out=qf, in0=qf, in1=qs2, op=ADD)

    return qf, rope_tail


def _emit_proj(ctx, w_sb, b_sb, x_tiles, tagpfx, io, ocmaj=False):
    """All 8 chunks of one projection, rope tails pipelined one chunk late."""
    outs = [None] * 8
    pend = None
    for oc in range(8):
        outs[oc], tail = _proj_chunk(ctx, w_sb, b_sb, x_tiles, tagpfx, oc, io,
                                     ocmaj=ocmaj)
        if pend is not None:
            pend()
        pend = tail
    if pend is not None:
        pend()
    return outs


def _emit_v_proj(ctx, xkv_tiles):
    """V projection into interleaved v_aug = [v_h | 1 x64] (128 cols/head)."""
    nc = ctx.nc
    for kc in range(4):
        va = ctx.vaug[kc]
        va3 = va.rearrange("p (h c) -> p h c", c=128)
        for b2 in range(2):
            ps = _bank(ctx)
            for c in range(8):
                nc.tensor.matmul(
                    ps,
                    lhsT=xkv_tiles[c][:, kc * 128 : (kc + 1) * 128],
                    rhs=ctx.wv_sb[c][:, b2 * 512 : (b2 + 1) * 512],
                    start=(c == 0),
                    stop=(c == 7),
                )
            nc.vector.tensor_copy(
                out=va3[:, b2 * 8 : (b2 + 1) * 8, 0:64],
                in_=ps.rearrange("p (h c) -> p h c", c=64),
            )


def _emit_scores_hg(ctx, qT, kT, qh, hg, pts):
    """ScoresT + exp for one 4-head group of one 256-token block.

    Both k-chunks of a head share one [128,512] PSUM bank (column
    halves) -> ONE exp per head.  Emission rotates PE quadrants (g)
    across consecutive matmuls so stationary loads overlap streaming.
    """
    nc = ctx.nc
    qsl = slice(qh * 256, (qh + 1) * 256)
    rc, pc = hg, 4 + hg
    banks = [_bank(ctx) for _ in range(4)]          # heads 4hg .. 4hg+3
    for kc in range(2):
        ksl = slice(qh * 256 + kc * 128, qh * 256 + (kc + 1) * 128)
        csl = slice(kc * 256, (kc + 1) * 256)
        for cch, first in ((rc, True), (pc, False)):
            for g in range(4):
                r0 = 32 * g
                nc.tensor.matmul(
                    banks[g][:, csl],
                    lhsT=kT[cch][r0 : r0 + 32, ksl],
                    rhs=qT[cch][r0 : r0 + 32, qsl],
                    start=first,
                    stop=not first,
                    tile_position=(r0, 0),
                )
    for g in range(4):
        h = 4 * hg + g
        pt = ctx.ptp.tile([128, BT], WDT, tag=f"pt{h}", name=f"pt{h}", bufs=2)
        nc.scalar.activation(out=pt, in_=banks[g], func=EXP)
        pts[h] = pt


def _emit_pv_hp(ctx, qh, hp, pts, attnT):
    """PV + table-free reciprocal + normalized evacuation, one head pair.

    Two heads share a [128,512] PSUM bank (column halves); their
    denominators (replicated on partitions 64:128 via the v_aug ones
    columns) are reciprocal'd as exp(-ln(x)) on ScalarE -- Ln and Exp
    share an activation table, so no table reloads.
    """
    nc = ctx.nc
    qsl = slice(qh * 256, (qh + 1) * 256)
    ps = _bank(ctx)
    for i in range(2):
        h = 2 * hp + i
        for kc in range(2):
            nc.tensor.matmul(
                ps[:, i * 256 : (i + 1) * 256],
                lhsT=ctx.vaug[qh * 2 + kc][:, h * 128 : (h + 1) * 128],
                rhs=pts[h][:, kc * 256 : (kc + 1) * 256],
                start=(kc == 0),
                stop=(kc == 1),
            )
    lnd = ctx.attnp.tile([64, BT], F32, tag="lnd", name="lnd")
    nc.scalar.activation(out=lnd, in_=ps[64:128, :], func=LN)
    rec = ctx.attnp.tile([64, BT], F32, tag="rec", name="rec", bufs=2)
    nc.scalar.activation(out=rec, in_=lnd, func=EXP, scale=-1.0)
    for i in range(2):
        h = 2 * hp + i
        cc, r0 = h // 2, (h % 2) * 64
        nc.vector.tensor_tensor(
            out=attnT[cc][r0 : r0 + 64, qsl],
            in0=ps[0:64, i * 256 : (i + 1) * 256],
            in1=rec[:, i * 256 : (i + 1) * 256],
            op=MULT,
        )


def _emit_outproj(ctx, attnT, pair, t2_range, evac="vector"):
    """Output projection + store for a subset of 128-token chunks.

    evac="scalar" routes the PSUM evacuation through ScalarE Identity
    (present in every activation table, so never a table load) -- used
    for chunks emitted while ScalarE has no exps pending.
    """
    nc = ctx.nc
    for t2 in t2_range:
        for n2 in range(2):
            ps = _bank(ctx)
            for cc in range(8):
                nc.tensor.matmul(
                    ps,
                    lhsT=attnT[cc][:, t2 * 128 : (t2 + 1) * 128],
                    rhs=ctx.wo_sb[cc][:, n2 * 512 : (n2 + 1) * 512],
                    start=(cc == 0),
                    stop=(cc == 7),
                )
            ob = ctx.outp.tile([128, 512], WDT, tag="outsb", name="outsb", bufs=2)
            if ctx.use_bias:
                nc.vector.tensor_tensor(
                    out=ob, in0=ps, in1=ctx.bo_sb[:, n2 * 512 : (n2 + 1) * 512],
                    op=ADD,
                )
            elif evac == "scalar":
                nc.scalar.activation(
                    out=ob, in_=ps,
                    func=mybir.ActivationFunctionType.Identity,
                )
            else:
                nc.vector.tensor_copy(out=ob, in_=ps)
            nc.sync.dma_start(
                out=ctx.out_d[
                    2 * pair + t2 // 2,
                    (t2 % 2) * 128 : (t2 % 2 + 1) * 128,
                    n2 * 512 : (n2 + 1) * 512,
                ],
                in_=ob,
            )


def build_program(use_bias):
    nc = bass.Bass("TRN2")
    ctx = _Ctx(nc, use_bias)
    ctx.xq_d = nc.dram_tensor("xq", [8, 128, TOK], WDT, kind="ExternalInput")
    ctx.xkv_d = nc.dram_tensor("xkv", [8, 128, TOK], WDT, kind="ExternalInput")
    wq_d = nc.dram_tensor("wq", [8, 128, F], WDT, kind="ExternalInput")
    wk_d = nc.dram_tensor("wk", [8, 128, F], WDT, kind="ExternalInput")
    wv_d = nc.dram_tensor("wv", [8, 128, F], WDT, kind="ExternalInput")
    wo_d = nc.dram_tensor("wo", [8, 128, F], WDT, kind="ExternalInput")
    rt_d = nc.dram_tensor("rt", [128, 128], WDT, kind="ExternalInput")
    ones_d = nc.dram_tensor("ones", [1, 16, 64], WDT, kind="ExternalInput")
    bq_d = nc.dram_tensor("bq", [128, 8], F32, kind="ExternalInput")
    bk_d = nc.dram_tensor("bk", [128, 8], F32, kind="ExternalInput")
    bo_d = nc.dram_tensor("bo", [1, F], F32, kind="ExternalInput")
    ctx.cos_d = nc.dram_tensor("cos", [NPAIR, 128, BT], F32, kind="ExternalInput")
    ctx.sin_d = nc.dram_tensor("sin", [NPAIR, 128, BT], F32, kind="ExternalInput")
    ctx.out_d = nc.dram_tensor("out", [BPC, BS, F], WDT, kind="ExternalOutput")

    with tile.TileContext(nc) as tc:
        with (
            tc.tile_pool(name="wpool", bufs=1) as wpool,
            tc.tile_pool(name="psum", bufs=8, space="PSUM") as psum,
            tc.tile_pool(name="xtp", bufs=2) as xtp,
            tc.tile_pool(name="qk", bufs=1) as qk,
            tc.tile_pool(name="ptp", bufs=1) as ptp,
            tc.tile_pool(name="attnp", bufs=1) as attnp,
            tc.tile_pool(name="outp", bufs=1) as outp,
            tc.tile_pool(name="tabp", bufs=1) as tabp,
        ):
            ctx.psum, ctx.xtp, ctx.qk = psum, xtp, qk
            ctx.ptp, ctx.attnp, ctx.outp, ctx.tabp = ptp, attnp, outp, tabp

            def wtiles(src, tagpfx):
                ts = []
                for c in range(8):
                    t = wpool.tile([128, F], WDT, tag=f"{tagpfx}{c}",
                                   name=f"{tagpfx}{c}")
                    nc.sync.dma_start(out=t, in_=src[c])
                    ts.append(t)
                return ts

            # prologue queue plan: wq split over sync+gpsimd (the first
            # projection trickles behind it), rt right behind (first rope
            # matmul), then the first pair's x spread over the three
            # queues, then the remaining weights in consumption order.
            ctx.wq_sb = [None] * 8
            for c in range(0, 8, 2):
                t = wpool.tile([128, F], WDT, tag=f"wq{c}", name=f"wq{c}")
                nc.sync.dma_start(out=t, in_=wq_d[c])
                ctx.wq_sb[c] = t
            ctx.rt_sb = wpool.tile([128, 128], WDT, tag="rt", name="rt")
            nc.sync.dma_start(out=ctx.rt_sb, in_=rt_d[:])
            io = _emit_x_dma(ctx, 0, prologue=True)
            for c in range(1, 8, 2):
                t = wpool.tile([128, F], WDT, tag=f"wq{c}", name=f"wq{c}")
                nc.gpsimd.dma_start(out=t, in_=wq_d[c])
                ctx.wq_sb[c] = t
            ctx.wk_sb = wtiles(wk_d, "wk")
            ctx.wv_sb = wtiles(wv_d, "wv")
            ctx.wo_sb = wtiles(wo_d, "wo")
            ctx.bq_sb = ctx.bk_sb = ctx.bo_sb = None
            if use_bias:
                ctx.bq_sb = wpool.tile([128, 8], F32, tag="bq", name="bq")
                ctx.bk_sb = wpool.tile([128, 8], F32, tag="bk", name="bk")
                ctx.bo_sb = wpool.tile([128, F], F32, tag="bo", name="bo")
                nc.sync.dma_start(out=ctx.bq_sb, in_=bq_d[:])
                nc.sync.dma_start(out=ctx.bk_sb, in_=bk_d[:])
                nc.sync.dma_start(
                    out=ctx.bo_sb, in_=bo_d[0:1, :].to_broadcast([128, F])
                )

            ctx.vaug = []
            for kc in range(4):
                va = wpool.tile([128, 2048], WDT, tag=f"vaug{kc}",
                                name=f"vaug{kc}")
                nc.sync.dma_start(
                    out=va.rearrange("p (h c) -> p h c", c=128)[:, :, 64:128],
                    in_=ones_d[:].to_broadcast([128, 16, 64]),
                )
                ctx.vaug.append(va)

            attnT = [
                attnp.tile([128, BT], WDT, tag=f"attnT{cc}", name=f"attnT{cc}")
                for cc in range(8)
            ]

            def proj_pair(io):
                qT = _emit_proj(ctx, ctx.wq_sb, ctx.bq_sb, io["xq"], "q", io, ocmaj=True)
                kT = _emit_proj(ctx, ctx.wk_sb, ctx.bk_sb, io["xkv"], "k", io)
                _emit_v_proj(ctx, io["xkv"])
                return qT, kT

            # ---- steady-state pipeline over pairs.  Interleave emission
            # so the PE never waits on ScalarE exps: block-1 scores fill
            # the exp lag of block-0 PV, and the output projection fills
            # the lag of block-1 PV.
            qT, kT = proj_pair(io)
            for pair in range(NPAIR):
                io_next = _emit_x_dma(ctx, pair + 1) if pair + 1 < NPAIR else None
                pts0, pts1 = {}, {}
                for hg in range(4):
                    _emit_scores_hg(ctx, qT, kT, 0, hg, pts0)
                for hg in range(4):
                    _emit_scores_hg(ctx, qT, kT, 1, hg, pts1)
                    _emit_pv_hp(ctx, 0, 2 * hg, pts0, attnT)
                    _emit_pv_hp(ctx, 0, 2 * hg + 1, pts0, attnT)
                for hg in range(4):
                    _emit_pv_hp(ctx, 1, 2 * hg, pts1, attnT)
                    _emit_pv_hp(ctx, 1, 2 * hg + 1, pts1, attnT)
                    if hg == 0:
                        _emit_outproj(ctx, attnT, pair, (0,))
                    elif hg == 2:
                        _emit_outproj(ctx, attnT, pair, (1,))
                if io_next is None:
                    _emit_outproj(ctx, attnT, pair, (2,), evac="scalar")
                    _emit_outproj(ctx, attnT, pair, (3,), evac="vector")
                else:
                    # defer the last out chunks behind the next pair's
                    # projections: they sit downstream of the whole
                    # ScalarE softmax chain, the projections do not.
                    qT = _emit_proj(ctx, ctx.wq_sb, ctx.bq_sb,
                                    io_next["xq"], "q", io_next, ocmaj=True)
                    _emit_outproj(ctx, attnT, pair, (2,), evac="scalar")
                    kT = _emit_proj(ctx, ctx.wk_sb, ctx.bk_sb,
                                    io_next["xkv"], "k", io_next)
                    _emit_outproj(ctx, attnT, pair, (3,), evac="scalar")
                    _emit_v_proj(ctx, io_next["xkv"])

    _split_multi_waits(nc)
    return nc


# ---------------------------------------------------------------- host side
def _host_prep(Wq, bq, Wk, bk, Wv, bv, Wo, bo):
    """Permute/scale weights; fold biases.

    Q/K output channels are permuted so rope dims occupy chunks 0-3
    (4 heads x 32 rope rows per chunk) and pass dims chunks 4-7: rope
    work (R-matmul + two combines) then touches only half the chunks.
    """
    old_of_new = np.empty(F, np.int64)
    for h in range(H):
        old_of_new[h * ROPE : (h + 1) * ROPE] = h * D + np.arange(ROPE)
        old_of_new[512 + h * ROPE : 512 + (h + 1) * ROPE] = (
            h * D + ROPE + np.arange(ROPE)
        )
    wq_flat = (Wq.reshape(F, F) / np.sqrt(D)).astype(np.float32)
    wq_p = np.ascontiguousarray(wq_flat[:, old_of_new]).reshape(8, 128, F)
    # re-block oc-major: wq2[oc][:, c*128:(c+1)*128] = wq_p[c][:, oc-block]
    wq_p = np.ascontiguousarray(
        wq_p.reshape(8, 128, 8, 128).transpose(2, 1, 0, 3).reshape(8, 128, F)
    )
    wk_flat = Wk.reshape(F, F).astype(np.float32)
    wk_p = np.ascontiguousarray(wk_flat[:, old_of_new]).reshape(8, 128, F)
    wv_c = np.ascontiguousarray(Wv.reshape(F, F)).reshape(8, 128, F)
    wo_c = np.ascontiguousarray(Wo.reshape(F, F)).reshape(8, 128, F)
    bq_p = np.ascontiguousarray(
        (bq.reshape(F) / np.sqrt(D))[old_of_new].reshape(8, 128).T
    ).astype(np.float32)
    bk_p = np.ascontiguousarray(
        bk.reshape(F)[old_of_new].reshape(8, 128).T
    ).astype(np.float32)
    bo_eff = (bo + bv.reshape(F) @ Wo.reshape(F, F)).reshape(1, F).astype(np.float32)

    # R^T for rotate_every_two with signs: (R@q)[2i] = -q[2i+1]; [2i+1] = q[2i]
    R = np.zeros((128, 128), np.float32)
    for g in range(4):          # 4 heads per rope chunk, 32 rows each
        for i in range(ROPE // 2):
            R[g * 32 + 2 * i, g * 32 + 2 * i + 1] = -1.0
            R[g * 32 + 2 * i + 1, g * 32 + 2 * i] = 1.0
    rt = np.ascontiguousarray(R.T)
    return wq_p, wk_p, wv_c, wo_c, bq_p, bk_p, bo_eff, rt


def _tables_for_core(core):
    """cos/sin tables [NPAIR, 128, 512] for this core's block pairs."""
    inv_freq = 1.0 / 10000.0 ** (np.arange(0, ROPE, 2) / ROPE)
    cos_t = np.empty((NPAIR, 128, BT), np.float32)
    sin_t = np.empty((NPAIR, 128, BT), np.float32)
    for p in range(NPAIR):
        for half in range(2):
            nb = (core * BPC + 2 * p + half) % NB
            pos = nb * BS + np.arange(BS, dtype=np.float64)
            ang = pos[None, :] * inv_freq[:, None]          # [16, 256]
            cpat = np.repeat(np.cos(ang), 2, axis=0)        # [32, 256]
            spat = np.repeat(np.sin(ang), 2, axis=0)
            sl = slice(half * BS, (half + 1) * BS)
            cos_t[p, :, sl] = np.tile(cpat, (4, 1))
            sin_t[p, :, sl] = np.tile(spat, (4, 1))
    return cos_t, sin_t


_nc_cache = {}


def kernel(inputs_q, inputs_kv, Wq, bq, Wk, bk, Wv, bv, Wo, bo):
    inputs_q = np.asarray(inputs_q, np.float32)
    inputs_kv = np.asarray(inputs_kv, np.float32)
    bq = np.asarray(bq)
    bk = np.asarray(bk)
    bv = np.asarray(bv)
    bo = np.asarray(bo)
    wq_p, wk_p, wv_c, wo_c, bq_p, bk_p, bo_eff, rt = _host_prep(
        np.asarray(Wq), bq, np.asarray(Wk), bk,
        np.asarray(Wv), bv, np.asarray(Wo), bo,
    )
    use_bias = bool(np.any(bq_p) or np.any(bk_p) or np.any(bo_eff))
    xq_all = inputs_q.reshape(BLKS, BS, F)
    xkv_all = inputs_kv.reshape(BLKS, BS, F)
    wq_p = wq_p.astype(WNP)
    wk_p = wk_p.astype(WNP)
    wv_c = wv_c.astype(WNP)
    wo_c = wo_c.astype(WNP)

    if use_bias not in _nc_cache:
        _nc_cache[use_bias] = build_program(use_bias)
    nc = _nc_cache[use_bias]

    in_maps = []
    for core in range(NCORES):
        cos_t, sin_t = _tables_for_core(core)
        # host-side x^T in bf16: [tokens, F] -> [F, tokens] -> [8,128,TOK]
        xq_c = xq_all[core * BPC : (core + 1) * BPC].reshape(TOK, F)
        xkv_c = xkv_all[core * BPC : (core + 1) * BPC].reshape(TOK, F)
        xq_t = np.ascontiguousarray(xq_c.T.astype(WNP)).reshape(8, 128, TOK)
        xkv_t = np.ascontiguousarray(xkv_c.T.astype(WNP)).reshape(8, 128, TOK)
        in_maps.append(
            {
                "xq": xq_t, "xkv": xkv_t,
                "wq": wq_p, "wk": wk_p, "wv": wv_c, "wo": wo_c,
                "rt": rt.astype(WNP), "bq": bq_p, "bk": bk_p, "bo": bo_eff,
                "ones": np.ones((1, 16, 64), WNP),
                "cos": cos_t, "sin": sin_t,
            }
        )
    res = run_bass_kernel_spmd(nc, in_maps, list(range(NCORES)))
    out = np.concatenate([res.results[i]["out"] for i in range(NCORES)], axis=0)
    return out.reshape(B, NB, BS, F).astype(np.float32)


# revision 24
# speedup vs baseline: 1.0107x; 1.0107x over previous
"""Trainium2 Bass kernel for nn_MultiHeadDotProductAttention_14980845928960.

Block-local multi-head attention with partial RoPE:
  q/k/v projections -> RoPE on first 32 of 64 head dims -> softmax(QK^T/8)V
  -> output projection.  Shapes: inputs [4,16,256,1024], 16 heads x 64 dim,
  blocks of 256 tokens attend locally.

Strategy: data-parallel over the 64 (batch, block) pairs -> 8 blocks/core,
processed as 4 pairs of blocks (512 tokens each).
  - x^T prepared HOST-side (transpose + bf16 cast) so no PE transposes or
    their PSUM evacuations are needed; DMA traffic for x halves.
  - Q/K channel-PERMUTED (host side) so rope dims occupy out-chunks 0-3
    and pass dims chunks 4-7; RoPE = R-matmul (pair swap w/ signs) + two
    elementwise multiplies with cos/sin tables; the R-matmul is emitted
    one chunk late (software pipeline) so the PE never waits on the DVE
    evacuation it consumes.
  - scores computed TRANSPOSED (k on partitions); both k-chunks of one
    head share a single [128,512] PSUM bank so ONE exp per head runs on
    ScalarE; softmax needs no max-subtraction (scores ~N(0,1)).
  - PV packs two heads per PSUM bank; denominators arrive replicated on
    PV-output partitions 64:128 via v_aug = [v_h | 1 x64]; reciprocal is
    exp(-ln(x)) on ScalarE -- both funcs live in ONE activation table so
    no ACT_TABLE_LOAD thrash; normalization folds into the PSUM->SBUF
    evacuation on DVE.
  - biases are all-zero per the problem spec; the fast path elides the
    bias adds entirely (a bias-capable slow path is kept for safety).
  - compute dtype bf16 on the PE, fp32 PSUM accumulate; output written
    bf16 and cast to f32 on host.
  - per-pair input DMA is prefetched one pair ahead on separate queues.
"""

import ml_dtypes
import numpy as np

import concourse.bass as bass
import concourse.tile as tile
from concourse import mybir
from concourse.bass_utils import run_bass_kernel_spmd

# ---------------------------------------------------------------- constants
B, NB, BS, F = 4, 16, 256, 1024
H, D, ROPE = 16, 64, 32
NCORES = 8
BLKS = B * NB                 # 64 blocks total
BPC = BLKS // NCORES          # 8 blocks per core
NPAIR = BPC // 2              # block pairs per core
BT = 2 * BS                   # tokens per pair (512)
TOK = BPC * BS                # tokens per core (2048)
F32 = mybir.dt.float32
BF16 = mybir.dt.bfloat16
WDT = BF16
WNP = ml_dtypes.bfloat16
MULT = mybir.AluOpType.mult
ADD = mybir.AluOpType.add
EXP = mybir.ActivationFunctionType.Exp
LN = mybir.ActivationFunctionType.Ln

# ------------------------------------------------- walrus multi-wait splitter
# This walrus build rejects >1 sync-wait per instruction on several
# instruction structs. Tile attaches several waits to one instruction;
# hoist extras onto NOPs inserted just before it on the same engine.
_split_ctr = [0]


def _split_multi_waits(nc, maxw=1):
    for f in nc.m.functions:
        for bb in f.blocks:
            insts = list(bb.instructions)
            out = []
            changed = False
            for inst in insts:
                si = inst.sync_info
                waits = list(si.on_wait) if si and si.on_wait else []
                if len(waits) > maxw:
                    changed = True
                    for w in waits[:-maxw]:
                        _split_ctr[0] += 1
                        nop = mybir.InstNoOp(
                            name=f"wsplit-{_split_ctr[0]}",
                            ins=[],
                            outs=[],
                            engine=inst.engine,
                        )
                        nop.sync_info = mybir.SyncInfo(on_wait=[w], on_update=[])
                        nc.register_instruction(nop)
                        out.append(nop)
                    si.on_wait = waits[-maxw:]
                out.append(inst)
            if changed:
                bb.instructions = out


# ---------------------------------------------------------------- bass build
class _Ctx:
    """Per-program emission state (pools, weight tiles, dram handles)."""

    def __init__(self, nc, use_bias):
        self.nc = nc
        self.use_bias = use_bias


def _bank(ctx):
    """Allocate a full PSUM bank [128, 512] f32 from the shared FIFO ring."""
    return ctx.psum.tile([128, BT], F32, tag="bank", name="bank")


def _emit_x_dma(ctx, pair, prologue=False):
    """Issue x^T chunk loads + cos/sin tables for one pair (prefetch).

    In the prologue the first pair's xq is split across the gpsimd and
    scalar queues (wq owns the sync queue) so the first projection can
    start as early as possible.
    """
    nc = ctx.nc
    xq_t, xkv_t = [], []
    sl = slice(pair * BT, (pair + 1) * BT)
    for c in range(8):
        t = ctx.xtp.tile([128, BT], WDT, tag=f"xq{c}", name=f"xq{c}", bufs=2)
        eng = (nc.gpsimd if c % 2 == 0 else nc.scalar) if prologue else nc.gpsimd
        eng.dma_start(out=t, in_=ctx.xq_d[c][:, sl])
        xq_t.append(t)
    for c in range(8):
        t = ctx.xtp.tile([128, BT], WDT, tag=f"xk{c}", name=f"xk{c}", bufs=2)
        (nc.gpsimd if (prologue and c % 2) else nc.sync).dma_start(
            out=t, in_=ctx.xkv_d[c][:, sl])
        xkv_t.append(t)
    cos_sb = ctx.tabp.tile([128, BT], F32, tag="cos", name="cos", bufs=2)
    nc.scalar.dma_start(out=cos_sb, in_=ctx.cos_d[pair])
    sin_sb = ctx.tabp.tile([128, BT], F32, tag="sin", name="sin", bufs=2)
    nc.scalar.dma_start(out=sin_sb, in_=ctx.sin_d[pair])
    return dict(xq=xq_t, xkv=xkv_t, cos=cos_sb, sin=sin_sb)


def _proj_chunk(ctx, w_sb, b_sb, x_tiles, tagpfx, oc, io):
    """One 128-chan projection chunk: 8 accumulating matmuls + evacuation.

    Returns (qf_tile, rope_tail).  For rope chunks (oc<4) rope_tail is a
    closure finishing RoPE (R-matmul + cos/sin combines); the caller
    emits it one chunk later so the PE never waits on the DVE copy.
    """
    nc = ctx.nc
    ps = _bank(ctx)
    for c in range(8):
        nc.tensor.matmul(
            ps,
            lhsT=w_sb[c][:, oc * 128 : (oc + 1) * 128],
            rhs=x_tiles[c],
            start=(c == 0),
            stop=(c == 7),
        )
    qf = ctx.qk.tile([128, BT], WDT, tag=f"{tagpfx}{oc}", name=f"{tagpfx}{oc}")
    if oc >= 4:
        # pass chunk: plain evacuation (bias add only on slow path).
        # GPSIMD cannot read PSUM, so this stays on DVE.
        if ctx.use_bias:
            nc.vector.tensor_scalar_add(qf, ps, b_sb[:, oc : oc + 1])
        else:
            nc.vector.tensor_copy(out=qf, in_=ps)
        return qf, None

    raw = ctx.qk.tile([128, BT], WDT, tag=f"{tagpfx}r{oc}", name=f"{tagpfx}r{oc}")
    if ctx.use_bias:
        nc.vector.tensor_scalar_add(raw, ps, b_sb[:, oc : oc + 1])
    else:
        nc.vector.tensor_copy(out=raw, in_=ps)

    def rope_tail():
        ps2 = _bank(ctx)
        nc.tensor.matmul(ps2, lhsT=ctx.rt_sb, rhs=raw, start=True, stop=True)
        qs2 = ctx.qk.tile([128, BT], F32, tag="qs2", name="qs2", bufs=2)
        nc.vector.tensor_tensor(out=qs2, in0=ps2, in1=io["sin"], op=MULT)
        nc.gpsimd.tensor_tensor(out=qf, in0=raw, in1=io["cos"], op=MULT)
        nc.gpsimd.tensor_tensor(out=qf, in0=qf, in1=qs2, op=ADD)

    return qf, rope_tail


def _emit_proj(ctx, w_sb, b_sb, x_tiles, tagpfx, io):
    """All 8 chunks of one projection, rope tails pipelined one chunk late."""
    outs = [None] * 8
    pend = None
    for oc in range(8):
        outs[oc], tail = _proj_chunk(ctx, w_sb, b_sb, x_tiles, tagpfx, oc, io)
        if pend is not None:
            pend()
        pend = tail
    if pend is not None:
        pend()
    return outs


def _emit_v_proj(ctx, xkv_tiles):
    """V projection into interleaved v_aug = [v_h | 1 x64] (128 cols/head)."""
    nc = ctx.nc
    for kc in range(4):
        va = ctx.vaug[kc]
        va3 = va.rearrange("p (h c) -> p h c", c=128)
        for b2 in range(2):
            ps = _bank(ctx)
            for c in range(8):
                nc.tensor.matmul(
                    ps,
                    lhsT=xkv_tiles[c][:, kc * 128 : (kc + 1) * 128],
                    rhs=ctx.wv_sb[c][:, b2 * 512 : (b2 + 1) * 512],
                    start=(c == 0),
                    stop=(c == 7),
                )
            nc.vector.tensor_copy(
                out=va3[:, b2 * 8 : (b2 + 1) * 8, 0:64],
                in_=ps.rearrange("p (h c) -> p h c", c=64),
            )


def _emit_scores_hg(ctx, qT, kT, qh, hg, pts):
    """ScoresT + exp for one 4-head group of one 256-token block.

    Both k-chunks of a head share one [128,512] PSUM bank (column
    halves) -> ONE exp per head.  Emission rotates PE quadrants (g)
    across consecutive matmuls so stationary loads overlap streaming.
    """
    nc = ctx.nc
    qsl = slice(qh * 256, (qh + 1) * 256)
    rc, pc = hg, 4 + hg
    banks = [_bank(ctx) for _ in range(4)]          # heads 4hg .. 4hg+3
    for kc in range(2):
        ksl = slice(qh * 256 + kc * 128, qh * 256 + (kc + 1) * 128)
        csl = slice(kc * 256, (kc + 1) * 256)
        for cch, first in ((rc, True), (pc, False)):
            for g in range(4):
                r0 = 32 * g
                nc.tensor.matmul(
                    banks[g][:, csl],
                    lhsT=kT[cch][r0 : r0 + 32, ksl],
                    rhs=qT[cch][r0 : r0 + 32, qsl],
                    start=first,
                    stop=not first,
                    tile_position=(r0, 0),
                )
    for g in range(4):
        h = 4 * hg + g
        pt = ctx.ptp.tile([128, BT], WDT, tag=f"pt{h}", name=f"pt{h}", bufs=2)
        nc.scalar.activation(out=pt, in_=banks[g], func=EXP)
        pts[h] = pt


def _emit_pv_hp(ctx, qh, hp, pts, attnT):
    """PV + table-free reciprocal + normalized evacuation, one head pair.

    Two heads share a [128,512] PSUM bank (column halves); their
    denominators (replicated on partitions 64:128 via the v_aug ones
    columns) are reciprocal'd as exp(-ln(x)) on ScalarE -- Ln and Exp
    share an activation table, so no table reloads.
    """
    nc = ctx.nc
    qsl = slice(qh * 256, (qh + 1) * 256)
    ps = _bank(ctx)
    for i in range(2):
        h = 2 * hp + i
        for kc in range(2):
            nc.tensor.matmul(
                ps[:, i * 256 : (i + 1) * 256],
                lhsT=ctx.vaug[qh * 2 + kc][:, h * 128 : (h + 1) * 128],
                rhs=pts[h][:, kc * 256 : (kc + 1) * 256],
                start=(kc == 0),
                stop=(kc == 1),
            )
    lnd = ctx.attnp.tile([64, BT], F32, tag="lnd", name="lnd")
    nc.scalar.activation(out=lnd, in_=ps[64:128, :], func=LN)
    rec = ctx.attnp.tile([64, BT], F32, tag="rec", name="rec", bufs=2)
    nc.scalar.activation(out=rec, in_=lnd, func=EXP, scale=-1.0)
    for i in range(2):
        h = 2 * hp + i
        cc, r0 = h // 2, (h % 2) * 64
        nc.vector.tensor_tensor(
            out=attnT[cc][r0 : r0 + 64, qsl],
            in0=ps[0:64, i * 256 : (i + 1) * 256],
            in1=rec[:, i * 256 : (i + 1) * 256],
            op=MULT,
        )


def _emit_outproj(ctx, attnT, pair, t2_range, evac="vector"):
    """Output projection + store for a subset of 128-token chunks.

    evac="scalar" routes the PSUM evacuation through ScalarE Identity
    (present in every activation table, so never a table load) -- used
    for chunks emitted while ScalarE has no exps pending.
    """
    nc = ctx.nc
    for t2 in t2_range:
        for n2 in range(2):
            ps = _bank(ctx)
            for cc in range(8):
                nc.tensor.matmul(
                    ps,
                    lhsT=attnT[cc][:, t2 * 128 : (t2 + 1) * 128],
                    rhs=ctx.wo_sb[cc][:, n2 * 512 : (n2 + 1) * 512],
                    start=(cc == 0),
                    stop=(cc == 7),
                )
            ob = ctx.outp.tile([128, 512], WDT, tag="outsb", name="outsb", bufs=2)
            if ctx.use_bias:
                nc.vector.tensor_tensor(
                    out=ob, in0=ps, in1=ctx.bo_sb[:, n2 * 512 : (n2 + 1) * 512],
                    op=ADD,
                )
            elif evac == "scalar":
                nc.scalar.activation(
                    out=ob, in_=ps,
                    func=mybir.ActivationFunctionType.Identity,
                )
            else:
                nc.vector.tensor_copy(out=ob, in_=ps)
            nc.sync.dma_start(
                out=ctx.out_d[
                    2 * pair + t2 // 2,
                    (t2 % 2) * 128 : (t2 % 2 + 1) * 128,
                    n2 * 512 : (n2 + 1) * 512,
                ],
                in_=ob,
            )


def build_program(use_bias):
    nc = bass.Bass("TRN2")
    ctx = _Ctx(nc, use_bias)
    ctx.xq_d = nc.dram_tensor("xq", [8, 128, TOK], WDT, kind="ExternalInput")
    ctx.xkv_d = nc.dram_tensor("xkv", [8, 128, TOK], WDT, kind="ExternalInput")
    wq_d = nc.dram_tensor("wq", [8, 128, F], WDT, kind="ExternalInput")
    wk_d = nc.dram_tensor("wk", [8, 128, F], WDT, kind="ExternalInput")
    wv_d = nc.dram_tensor("wv", [8, 128, F], WDT, kind="ExternalInput")
    wo_d = nc.dram_tensor("wo", [8, 128, F], WDT, kind="ExternalInput")
    rt_d = nc.dram_tensor("rt", [128, 128], WDT, kind="ExternalInput")
    ones_d = nc.dram_tensor("ones", [1, 16, 64], WDT, kind="ExternalInput")
    bq_d = nc.dram_tensor("bq", [128, 8], F32, kind="ExternalInput")
    bk_d = nc.dram_tensor("bk", [128, 8], F32, kind="ExternalInput")
    bo_d = nc.dram_tensor("bo", [1, F], F32, kind="ExternalInput")
    ctx.cos_d = nc.dram_tensor("cos", [NPAIR, 128, BT], F32, kind="ExternalInput")
    ctx.sin_d = nc.dram_tensor("sin", [NPAIR, 128, BT], F32, kind="ExternalInput")
    ctx.out_d = nc.dram_tensor("out", [BPC, BS, F], WDT, kind="ExternalOutput")

    with tile.TileContext(nc) as tc:
        with (
            tc.tile_pool(name="wpool", bufs=1) as wpool,
            tc.tile_pool(name="psum", bufs=8, space="PSUM") as psum,
            tc.tile_pool(name="xtp", bufs=2) as xtp,
            tc.tile_pool(name="qk", bufs=1) as qk,
            tc.tile_pool(name="ptp", bufs=1) as ptp,
            tc.tile_pool(name="attnp", bufs=1) as attnp,
            tc.tile_pool(name="outp", bufs=1) as outp,
            tc.tile_pool(name="tabp", bufs=1) as tabp,
        ):
            ctx.psum, ctx.xtp, ctx.qk = psum, xtp, qk
            ctx.ptp, ctx.attnp, ctx.outp, ctx.tabp = ptp, attnp, outp, tabp

            def wtiles(src, tagpfx):
                ts = []
                for c in range(8):
                    t = wpool.tile([128, F], WDT, tag=f"{tagpfx}{c}",
                                   name=f"{tagpfx}{c}")
                    nc.sync.dma_start(out=t, in_=src[c])
                    ts.append(t)
                return ts

            # prologue queue plan: wq split over sync+gpsimd (the first
            # projection trickles behind it), rt right behind (first rope
            # matmul), then the first pair's x spread over the three
            # queues, then the remaining weights in consumption order.
            ctx.wq_sb = []
            for c in range(8):
                t = wpool.tile([128, F], WDT, tag=f"wq{c}", name=f"wq{c}")
                (nc.sync if c % 2 == 0 else nc.gpsimd).dma_start(
                    out=t, in_=wq_d[c])
                ctx.wq_sb.append(t)
            ctx.rt_sb = wpool.tile([128, 128], WDT, tag="rt", name="rt")
            nc.sync.dma_start(out=ctx.rt_sb, in_=rt_d[:])
            io = _emit_x_dma(ctx, 0, prologue=True)
            ctx.wk_sb = wtiles(wk_d, "wk")
            ctx.wv_sb = wtiles(wv_d, "wv")
            ctx.wo_sb = wtiles(wo_d, "wo")
            ctx.bq_sb = ctx.bk_sb = ctx.bo_sb = None
            if use_bias:
                ctx.bq_sb = wpool.tile([128, 8], F32, tag="bq", name="bq")
                ctx.bk_sb = wpool.tile([128, 8], F32, tag="bk", name="bk")
                ctx.bo_sb = wpool.tile([128, F], F32, tag="bo", name="bo")
                nc.sync.dma_start(out=ctx.bq_sb, in_=bq_d[:])
                nc.sync.dma_start(out=ctx.bk_sb, in_=bk_d[:])
                nc.sync.dma_start(
                    out=ctx.bo_sb, in_=bo_d[0:1, :].to_broadcast([128, F])
                )

            ctx.vaug = []
            for kc in range(4):
                va = wpool.tile([128, 2048], WDT, tag=f"vaug{kc}",
                                name=f"vaug{kc}")
                nc.sync.dma_start(
                    out=va.rearrange("p (h c) -> p h c", c=128)[:, :, 64:128],
                    in_=ones_d[:].to_broadcast([128, 16, 64]),
                )
                ctx.vaug.append(va)

            attnT = [
                attnp.tile([128, BT], WDT, tag=f"attnT{cc}", name=f"attnT{cc}")
                for cc in range(8)
            ]

            def proj_pair(io):
                qT = _emit_proj(ctx, ctx.wq_sb, ctx.bq_sb, io["xq"], "q", io)
                kT = _emit_proj(ctx, ctx.wk_sb, ctx.bk_sb, io["xkv"], "k", io)
                _emit_v_proj(ctx, io["xkv"])
                return qT, kT

            # ---- steady-state pipeline over pairs.  Interleave emission
            # so the PE never waits on ScalarE exps: block-1 scores fill
            # the exp lag of block-0 PV, and the output projection fills
            # the lag of block-1 PV.
            qT, kT = proj_pair(io)
            for pair in range(NPAIR):
                io_next = _emit_x_dma(ctx, pair + 1) if pair + 1 < NPAIR else None
                pts0, pts1 = {}, {}
                for hg in range(4):
                    _emit_scores_hg(ctx, qT, kT, 0, hg, pts0)
                for hg in range(4):
                    _emit_scores_hg(ctx, qT, kT, 1, hg, pts1)
                    _emit_pv_hp(ctx, 0, 2 * hg, pts0, attnT)
                    _emit_pv_hp(ctx, 0, 2 * hg + 1, pts0, attnT)
                for hg in range(4):
                    _emit_pv_hp(ctx, 1, 2 * hg, pts1, attnT)
                    _emit_pv_hp(ctx, 1, 2 * hg + 1, pts1, attnT)
                    if hg == 0:
                        _emit_outproj(ctx, attnT, pair, (0,))
                    elif hg == 2:
                        _emit_outproj(ctx, attnT, pair, (1,))
                if io_next is None:
                    _emit_outproj(ctx, attnT, pair, (2, 3), evac="scalar")
                else:
                    # defer the last out chunks behind the next pair's
                    # projections: they sit downstream of the whole
                    # ScalarE softmax chain, the projections do not.
                    qT = _emit_proj(ctx, ctx.wq_sb, ctx.bq_sb,
                                    io_next["xq"], "q", io_next)
                    _emit_outproj(ctx, attnT, pair, (2,), evac="scalar")
                    kT = _emit_proj(ctx, ctx.wk_sb, ctx.bk_sb,
                                    io_next["xkv"], "k", io_next)
                    _emit_outproj(ctx, attnT, pair, (3,), evac="scalar")
                    _emit_v_proj(ctx, io_next["xkv"])

    _split_multi_waits(nc)
    return nc


# ---------------------------------------------------------------- host side
def _host_prep(Wq, bq, Wk, bk, Wv, bv, Wo, bo):
    """Permute/scale weights; fold biases.

    Q/K output channels are permuted so rope dims occupy chunks 0-3
    (4 heads x 32 rope rows per chunk) and pass dims chunks 4-7: rope
    work (R-matmul + two combines) then touches only half the chunks.
    """
    old_of_new = np.empty(F, np.int64)
    for h in range(H):
        old_of_new[h * ROPE : (h + 1) * ROPE] = h * D + np.arange(ROPE)
        old_of_new[512 + h * ROPE : 512 + (h + 1) * ROPE] = (
            h * D + ROPE + np.arange(ROPE)
        )
    wq_flat = (Wq.reshape(F, F) / np.sqrt(D)).astype(np.float32)
    wq_p = np.ascontiguousarray(wq_flat[:, old_of_new]).reshape(8, 128, F)
    wk_flat = Wk.reshape(F, F).astype(np.float32)
    wk_p = np.ascontiguousarray(wk_flat[:, old_of_new]).reshape(8, 128, F)
    wv_c = np.ascontiguousarray(Wv.reshape(F, F)).reshape(8, 128, F)
    wo_c = np.ascontiguousarray(Wo.reshape(F, F)).reshape(8, 128, F)
    bq_p = np.ascontiguousarray(
        (bq.reshape(F) / np.sqrt(D))[old_of_new].reshape(8, 128).T
    ).astype(np.float32)
    bk_p = np.ascontiguousarray(
        bk.reshape(F)[old_of_new].reshape(8, 128).T
    ).astype(np.float32)
    bo_eff = (bo + bv.reshape(F) @ Wo.reshape(F, F)).reshape(1, F).astype(np.float32)

    # R^T for rotate_every_two with signs: (R@q)[2i] = -q[2i+1]; [2i+1] = q[2i]
    R = np.zeros((128, 128), np.float32)
    for g in range(4):          # 4 heads per rope chunk, 32 rows each
        for i in range(ROPE // 2):
            R[g * 32 + 2 * i, g * 32 + 2 * i + 1] = -1.0
            R[g * 32 + 2 * i + 1, g * 32 + 2 * i] = 1.0
    rt = np.ascontiguousarray(R.T)
    return wq_p, wk_p, wv_c, wo_c, bq_p, bk_p, bo_eff, rt


def _tables_for_core(core):
    """cos/sin tables [NPAIR, 128, 512] for this core's block pairs."""
    inv_freq = 1.0 / 10000.0 ** (np.arange(0, ROPE, 2) / ROPE)
    cos_t = np.empty((NPAIR, 128, BT), np.float32)
    sin_t = np.empty((NPAIR, 128, BT), np.float32)
    for p in range(NPAIR):
        for half in range(2):
            nb = (core * BPC + 2 * p + half) % NB
            pos = nb * BS + np.arange(BS, dtype=np.float64)
            ang = pos[None, :] * inv_freq[:, None]          # [16, 256]
            cpat = np.repeat(np.cos(ang), 2, axis=0)        # [32, 256]
            spat = np.repeat(np.sin(ang), 2, axis=0)
            sl = slice(half * BS, (half + 1) * BS)
            cos_t[p, :, sl] = np.tile(cpat, (4, 1))
            sin_t[p, :, sl] = np.tile(spat, (4, 1))
    return cos_t, sin_t


_nc_cache = {}


def kernel(inputs_q, inputs_kv, Wq, bq, Wk, bk, Wv, bv, Wo, bo):
    inputs_q = np.asarray(inputs_q, np.float32)
    inputs_kv = np.asarray(inputs_kv, np.float32)
    bq = np.asarray(bq)
    bk = np.asarray(bk)
    bv = np.asarray(bv)
    bo = np.asarray(bo)
    wq_p, wk_p, wv_c, wo_c, bq_p, bk_p, bo_eff, rt = _host_prep(
        np.asarray(Wq), bq, np.asarray(Wk), bk,
        np.asarray(Wv), bv, np.asarray(Wo), bo,
    )
    use_bias = bool(np.any(bq_p) or np.any(bk_p) or np.any(bo_eff))
    xq_all = inputs_q.reshape(BLKS, BS, F)
    xkv_all = inputs_kv.reshape(BLKS, BS, F)
    wq_p = wq_p.astype(WNP)
    wk_p = wk_p.astype(WNP)
    wv_c = wv_c.astype(WNP)
    wo_c = wo_c.astype(WNP)

    if use_bias not in _nc_cache:
        _nc_cache[use_bias] = build_program(use_bias)
    nc = _nc_cache[use_bias]

    in_maps = []
    for core in range(NCORES):
        cos_t, sin_t = _tables_for_core(core)
        # host-side x^T in bf16: [tokens, F] -> [F, tokens] -> [8,128,TOK]
        xq_c = xq_all[core * BPC : (core + 1) * BPC].reshape(TOK, F)
        xkv_c = xkv_all[core * BPC : (core + 1) * BPC].reshape(TOK, F)
        xq_t = np.ascontiguousarray(xq_c.T.astype(WNP)).reshape(8, 128, TOK)
        xkv_t = np.ascontiguousarray(xkv_c.T.astype(WNP)).reshape(8, 128, TOK)
        in_maps.append(
            {
                "xq": xq_t, "xkv": xkv_t,
                "wq": wq_p, "wk": wk_p, "wv": wv_c, "wo": wo_c,
                "rt": rt.astype(WNP), "bq": bq_p, "bk": bk_p, "bo": bo_eff,
                "ones": np.ones((1, 16, 64), WNP),
                "cos": cos_t, "sin": sin_t,
            }
        )
    res = run_bass_kernel_spmd(nc, in_maps, list(range(NCORES)))
    out = np.concatenate([res.results[i]["out"] for i in range(NCORES)], axis=0)
    return out.reshape(B, NB, BS, F).astype(np.float32)
